# revision 1
# baseline (speedup 1.0000x reference)
"""Trainium2 Bass kernel for nn_Attention_27376121544790.

Math (per batch element, B=8 -> one element per NeuronCore, no collectives):
  qk   = (x + gamma*pos_flat) @ W.T + b          [N, D]
  q = k = l2norm(qk per 64-dim head)
  S    = (q @ k.T) * (sqrt(64)/attn_gamma)       per head, SYMMETRIC
  attn = softmax(S)  (logits in [-.8,.8] at the shipped attn_gamma=10 -> no
         max-subtraction needed; E = exp(S) stays symmetric so E tiles are
         reused untransposed as the second matmul's moving operand)
  out  = attn @ v,  v = x head-split
  final= (w0*out + w1*x) @ W.T + b,  w_i = exp(sum_gamma_i)/sum

Everything on-device lives in transposed [feature, token] layout so the
shared projection weight W.T serves both matmuls and no transposes are
needed; the host transposes the per-core result back.

Softmax denominators: v is augmented with a ones column per head, so the
attn@v matmul's extra output row is Z = sum_j E[j, n] (column sums = row
sums by symmetry). Row-vector -> half-partition broadcasts go through the
PE with a K=2 block-mask lhsT (bd2), since gpsimd partition_broadcast
drops the destination partition offset on HW and DVE lanes cannot cross
partitions.
"""

import math
import os

import numpy as np

B, N, C, D = 8, 1024, 1024, 1024
HEADS, HD = 16, 64
P = 128
EPS = 1e-6
NCHUNK = C // P  # 8 chunks of 128 feature rows
FH = 512         # free-dim half (matmul moving max for f32)


def _build(gamma: float, w0: float, w1: float, logit_scale: float):
    import concourse.bass as bass
    import concourse.tile as tile
    from concourse import bacc, mybir

    f32 = mybir.dt.float32
    MMDT = {
        "float32r": mybir.dt.float32r,
        "float32": mybir.dt.float32,
    }[os.environ.get("BK_MM_DTYPE", "float32r")]

    BF16 = mybir.dt.bfloat16

    def as32(ap):
        return ap.bitcast(f32) if MMDT != f32 else ap

    Exp = mybir.ActivationFunctionType.Exp
    Ln = mybir.ActivationFunctionType.Ln
    MULT = mybir.AluOpType.mult
    ADD = mybir.AluOpType.add

    nc = bacc.Bacc("TRN2", target_bir_lowering=False, debug=False)

    xT_d = nc.declare_dram_parameter("xT", [C, N], BF16, isOutput=False)
    xaug_d = nc.declare_dram_parameter("xaug", [N, HEADS * (HD + 1)], BF16, isOutput=False)
    WT_d = nc.declare_dram_parameter("WT", [C, D], BF16, isOutput=False)
    bmat_d = nc.declare_dram_parameter("bmat", [P, NCHUNK], f32, isOutput=False)
    bd_d = nc.declare_dram_parameter("bd", [C, HEADS], MMDT, isOutput=False)
    bd2_d = nc.declare_dram_parameter("bd2", [2, P], MMDT, isOutput=False)
    bd2w_d = nc.declare_dram_parameter("bd2w", [2, P], MMDT, isOutput=False)
    if gamma != 0.0:
        pos_d = nc.declare_dram_parameter("posCN", [C, N], f32, isOutput=False)
    out_d = nc.declare_dram_parameter("out", [D, N], f32, isOutput=True)

    with tile.TileContext(nc) as tc:
        with (
            tc.tile_pool(name="persist", bufs=1) as pers,
            tc.tile_pool(name="small", bufs=1) as small,
        ):
            # ---- persistent SBUF residency ----
            xT_t = [pers.tile([P, N], BF16, tag=f"xT{c}", name=f"xT{c}") for c in range(NCHUNK)]
            xaug_t = [pers.tile([P, HEADS * (HD + 1)], BF16, tag=f"xa{c}", name=f"xa{c}") for c in range(NCHUNK)]
            WT_t = [pers.tile([P, D], BF16, tag=f"WT{c}", name=f"WT{c}") for c in range(NCHUNK)]
            qkT_t = [pers.tile([P, N], MMDT, tag=f"qk{c}", name=f"qk{c}") for c in range(NCHUNK)]
            bd_t = [small.tile([P, HEADS], MMDT, tag=f"bd{c}", name=f"bd{c}") for c in range(NCHUNK)]
            bmat_t = small.tile([P, NCHUNK], f32, tag="bmat")
            bd2_t = small.tile([2, P], MMDT, tag="bd2")
            bd2w_t = small.tile([2, P], MMDT, tag="bd2w")
            dummy_t = small.tile([1, 16], f32, tag="dummy")
            invn_t = small.tile([HEADS, N], f32, tag="invn")
            invn_r = small.tile([HEADS, N], MMDT, tag="invnr")
            scr_t = small.tile([HEADS, N], f32, tag="scr")
            Zall_t = small.tile([HEADS, N], MMDT, tag="Zall")
            rZ_t = small.tile([HEADS, N], f32, tag="rZ")
            rZ_r = small.tile([HEADS, N], MMDT, tag="rZr")
            eps_t = small.tile([HEADS, 1], f32, tag="eps")

            # dummy Sqrt at t0 preloads the sqrt ACT table set during the
            # input-DMA wait; a matching dummy Exp is placed after the real
            # sqrt (data-dependent on scr_t so the scheduler keeps order).
            if os.environ.get("BK_DUMMY", "1") == "1":
                nc.gpsimd.memset(dummy_t[:], 1.0)
                nc.scalar.activation(dummy_t[:], dummy_t[:], Ln)

            for c in range(NCHUNK):
                nc.sync.dma_start(xT_t[c][:], xT_d[c * P:(c + 1) * P, :])
                nc.sync.dma_start(WT_t[c][:], WT_d[c * P:(c + 1) * P, :])
                nc.sync.dma_start(bd_t[c][:], bd_d[c * P:(c + 1) * P, :])
            nc.sync.dma_start(bmat_t[:], bmat_d[:])
            nc.sync.dma_start(bd2_t[:], bd2_d[:])
            nc.sync.dma_start(bd2w_t[:], bd2w_d[:])
            for c in range(NCHUNK):
                nc.sync.dma_start(xaug_t[c][:], xaug_d[c * P:(c + 1) * P, :])

            # ---- phase 0/1: (xp = x + gamma*pos) and qkT projection ----
            if gamma != 0.0:
                with tc.tile_pool(name="xp", bufs=1) as pxp, \
                     tc.tile_pool(name="posb", bufs=2) as ppos:
                    xpT_t = [pxp.tile([P, N], BF16, tag=f"xp{c}", name=f"xp{c}") for c in range(NCHUNK)]
                    for c in range(NCHUNK):
                        pt = ppos.tile([P, N], f32, tag="pos")
                        nc.sync.dma_start(pt[:], pos_d[c * P:(c + 1) * P, :])
                        nc.vector.scalar_tensor_tensor(
                            out=xpT_t[c][:], in0=pt[:], scalar=float(gamma),
                            in1=xT_t[c][:], op0=MULT, op1=ADD)
                    _proj_qkT(nc, tc, f32, xpT_t, WT_t, bmat_t, qkT_t)
            else:
                _proj_qkT(nc, tc, f32, xT_t, WT_t, bmat_t, qkT_t)

            # ---- phase 2: per-head inverse norms ----
            with (
                tc.tile_pool(name="sq", bufs=2) as psq,
                tc.tile_pool(name="psum_ssq", bufs=1, space="PSUM") as pssq,
            ):
                ps = pssq.tile([HEADS, N], f32, tag="ssq")
                for c in range(NCHUNK):
                    sq = psq.tile([P, N], MMDT, tag="sq")
                    nc.vector.tensor_mul(sq[:], qkT_t[c][:], qkT_t[c][:])
                    for fn in range(2):
                        nc.tensor.matmul(
                            ps[:, fn * FH:(fn + 1) * FH],
                            bd_t[c][:],
                            sq[:, fn * FH:(fn + 1) * FH],
                            start=(c == 0), stop=(c == NCHUNK - 1))
                # invn = 1/sqrt(ssq + eps); Sqrt on ACT, recip on DVE
                nc.gpsimd.memset(eps_t[:], EPS)
                nc.scalar.activation(scr_t[:], ps[:], Ln, bias=eps_t[:])
                nc.scalar.activation(invn_t[:], scr_t[:], Exp, scale=-0.5)
                nc.vector.tensor_copy(invn_r[:], invn_t[:])

            # ---- phases 3-5 share one psum pool set: the K=2 broadcast
            # matmuls (normalize + blend) borrow gram-pool slots so there is
            # no pool-swap barrier between normalization, attention and blend.
            head_order = list(range(0, HEADS, 2)) + list(range(1, HEADS, 2))
            with (
                tc.tile_pool(name="psum_g", bufs=3, space="PSUM") as pg_pool,
                tc.tile_pool(name="psum_av", bufs=2, space="PSUM") as pav_pool,
                tc.tile_pool(name="E", bufs=12) as pE,
                tc.tile_pool(name="avstage", bufs=2) as pstage,
                tc.tile_pool(name="pair", bufs=2) as ppair,
            ):
                # phase 3: qnT = qkT * bcast(invn), in place
                for c in range(NCHUNK):
                    pr = ppair.tile([2, N], MMDT, tag="pr")
                    nc.sync.dma_start(pr[:], invn_r[2 * c:2 * c + 2, :])
                    pbt = pg_pool.tile([P, N], f32, tag="pg", name="pbt")
                    for fn in range(2):
                        nc.tensor.matmul(
                            pbt[:, fn * FH:(fn + 1) * FH], bd2_t[:],
                            pr[0:2, fn * FH:(fn + 1) * FH], start=True, stop=True)
                    nc.vector.tensor_mul(qkT_t[c][:], qkT_t[c][:], pbt[:])

                # phase 4: attention per head
                for h in head_order:
                    c, half = h // 2, h % 2
                    qn_h = qkT_t[c][half * HD:(half + 1) * HD, :]
                    E_tiles = []
                    for mb in range(NCHUNK):
                        pg = pg_pool.tile([P, N], f32, tag="pg")
                        for fn in range(2):
                            nc.tensor.matmul(
                                pg[:, fn * FH:(fn + 1) * FH],
                                qn_h[:, mb * P:(mb + 1) * P],
                                qn_h[:, fn * FH:(fn + 1) * FH],
                                start=True, stop=True)
                        Et = pE.tile([P, N], BF16, tag="E")
                        nc.scalar.activation(Et[:], pg[:], Exp, scale=logit_scale)
                        E_tiles.append(Et)
                    stage = pstage.tile([HD + 1, N], MMDT, tag="stage")
                    for fn in range(2):
                        pav = pav_pool.tile([HD + 1, FH], f32, tag="pav")
                        for j in range(NCHUNK):
                            nc.tensor.matmul(
                                pav[:],
                                xaug_t[j][:, h * (HD + 1):(h + 1) * (HD + 1)],
                                E_tiles[j][:, fn * FH:(fn + 1) * FH],
                                start=(j == 0), stop=(j == NCHUNK - 1))
                        nc.vector.tensor_copy(
                            stage[:, fn * FH:(fn + 1) * FH], pav[:])
                    # out' into the dead qn_h rows; Z row into Zall[h]
                    # (DMA: engines cannot cross partitions, DMA can)
                    nc.sync.dma_start(Zall_t[h:h + 1, :], stage[HD:HD + 1, :])
                    nc.sync.dma_start(qn_h, stage[0:HD, :])

                # phase 5: blend B'' = w1*(xT + outT*bcast((w0/w1)/Z)) in
                # place on xT; the w1 factor rides the proj2 drain
                nc.vector.reciprocal_approx_fast(rZ_t[:], as32(Zall_t[:]))
                nc.vector.tensor_copy(rZ_r[:], rZ_t[:])
                for c in range(NCHUNK):
                    pr = ppair.tile([2, N], MMDT, tag="pr", name="zpr")
                    nc.sync.dma_start(pr[:], rZ_r[2 * c:2 * c + 2, :])
                    bz = pg_pool.tile([P, N], f32, tag="pg", name="bz")
                    for fn in range(2):
                        nc.tensor.matmul(
                            bz[:, fn * FH:(fn + 1) * FH], bd2w_t[:],
                            pr[0:2, fn * FH:(fn + 1) * FH], start=True, stop=True)
                    nc.vector.tensor_mul(qkT_t[c][:], qkT_t[c][:], bz[:])
                    nc.gpsimd.tensor_add(xT_t[c][:], xT_t[c][:], qkT_t[c][:])

            # ---- phase 6: final projection ----
            with (
                tc.tile_pool(name="psum_p2", bufs=2, space="PSUM") as pp2,
                tc.tile_pool(name="fin", bufs=2) as pfin,
            ):
                for m in range(NCHUNK):
                    fin = pfin.tile([P, N], f32, tag="fin")
                    for fn in range(2):
                        ps2 = pp2.tile([P, FH], f32, tag="p2")
                        for k in range(NCHUNK):
                            nc.tensor.matmul(
                                ps2[:],
                                WT_t[k][:, m * P:(m + 1) * P],
                                xT_t[k][:, fn * FH:(fn + 1) * FH],
                                start=(k == 0), stop=(k == NCHUNK - 1))
                        nc.vector.tensor_scalar(
                            fin[:, fn * FH:(fn + 1) * FH], ps2[:], float(w1),
                            bmat_t[:, m:m + 1], MULT, ADD)
                    nc.sync.dma_start(out_d[m * P:(m + 1) * P, :], fin[:])

    nc.compile()
    return nc


def _proj_qkT(nc, tc, f32, rhs_t, WT_t, bmat_t, qkT_t):
    """qkT[d, n] = sum_c WT[c, d] * xp[c, n] + b[d] (bias per partition)."""
    with tc.tile_pool(name="psum_p1", bufs=8, space="PSUM") as pp1:
        for m in range(NCHUNK):
            for fn in range(2):
                ps = pp1.tile([P, FH], f32, tag="p1")
                for k in range(NCHUNK):
                    nc.tensor.matmul(
                        ps[:],
                        WT_t[k][:, m * P:(m + 1) * P],
                        rhs_t[k][:, fn * FH:(fn + 1) * FH],
                        start=(k == 0), stop=(k == NCHUNK - 1))
                nc.vector.tensor_scalar_add(
                    qkT_t[m][:, fn * FH:(fn + 1) * FH], ps[:], bmat_t[:, m:m + 1])


def _host_prep(x, pos, W, b, gamma, w0, w1):
    """Per-core input shards (host layout work only)."""
    import ml_dtypes
    WT = np.ascontiguousarray(W.T).astype(ml_dtypes.bfloat16)  # [C, D]
    bmat = np.ascontiguousarray(b.reshape(NCHUNK, P).T)  # [P, 8], col m = b chunk m
    bd = np.zeros((C, HEADS), dtype=np.float32)          # block-diag ones
    for c in range(NCHUNK):
        bd[c * P:c * P + HD, 2 * c] = 1.0
        bd[c * P + HD:(c + 1) * P, 2 * c + 1] = 1.0
    bd2 = np.zeros((2, P), dtype=np.float32)             # half-partition masks
    bd2[0, :HD] = 1.0
    bd2[1, HD:] = 1.0
    bd2w = (bd2 * np.float32(w0 / w1)).astype(np.float32)  # w0/w1 into Z bcast; w1 in proj2 drain
    in_maps = []
    for i in range(B):
        xi = x[i]                                        # [N, C]
        import ml_dtypes
        xaug = np.empty((N, HEADS * (HD + 1)), dtype=ml_dtypes.bfloat16)
        for h in range(HEADS):
            xaug[:, h * (HD + 1):h * (HD + 1) + HD] = \
                xi[:, h * HD:(h + 1) * HD].astype(xaug.dtype)
            xaug[:, h * (HD + 1) + HD] = 1.0
        m = {
            "xT": np.ascontiguousarray(xi.T).astype(ml_dtypes.bfloat16),
            "xaug": xaug,
            "WT": WT,
            "bmat": bmat,
            "bd": bd,
            "bd2": bd2,
            "bd2w": bd2w,
        }
        if gamma != 0.0:
            m["posCN"] = np.ascontiguousarray(pos[i].reshape(C, N))
        in_maps.append(m)
    return in_maps


LAST_RESULT = None


def kernel(x, pos, W, b, gamma, attn_gamma, sum_gamma0, sum_gamma1):
    global LAST_RESULT
    from concourse.bass_utils import run_bass_kernel_spmd

    x = np.asarray(x, dtype=np.float32)
    pos = np.asarray(pos, dtype=np.float32)
    W = np.asarray(W, dtype=np.float32)
    b = np.asarray(b, dtype=np.float32)
    gamma = float(np.asarray(gamma))
    attn_gamma = float(np.asarray(attn_gamma))
    g0 = math.exp(float(np.asarray(sum_gamma0)))
    g1 = math.exp(float(np.asarray(sum_gamma1)))
    w0, w1 = g0 / (g0 + g1), g1 / (g0 + g1)
    logit_scale = math.sqrt(HD) / attn_gamma

    nc = _build(gamma, w0, w1, logit_scale)
    in_maps = _host_prep(x, pos, W, b, gamma, w0, w1)
    res = run_bass_kernel_spmd(
        nc, in_maps, core_ids=list(range(B)),
        trace=os.environ.get("BK_TRACE", "0") == "1",
    )
    LAST_RESULT = res
    out = np.empty((B, N, D), dtype=np.float32)
    for i in range(B):
        out[i] = res.results[i]["out"].T
    return out

